# revision 67
# baseline (speedup 1.0000x reference)
import sys, os
sys.path.insert(0, "/opt/trn_rl_repo")
import numpy as np
from contextlib import ExitStack

try:
    import ml_dtypes
    import concourse.bass as bass
    import concourse.mybir as mybir
    from concourse import tile
    from concourse.bass_utils import run_bass_kernel_spmd
    _HAVE_BASS = True
except Exception:
    _HAVE_BASS = False

BF16 = mybir.dt.bfloat16
F32 = mybir.dt.float32
F8 = mybir.dt.float8e4
AF = mybir.ActivationFunctionType
ALU = mybir.AluOpType
DR = mybir.MatmulPerfMode.DoubleRow

# geometry (hardcoded for this problem)
DIM = 48
HEADS = 8
CH = 6
B = 2
H = 256
W = 256
WP = W + 2          # padded width
R_IN = 68           # fp8 input rows per shard (64 owned + 2 halo each side)
R_MID = 66          # q/k/v, x' rows
R_OUT = 64          # owned output rows
NPIX_IN = R_IN * WP
NPIX_MID = R_MID * WP
QK_LEN = 64 * WP    # q/k computed only on rows 1..64 of the 66-row window
NT = 512            # matmul free-dim tile

# 3x3 tap pairing for fp8 DoubleRow (plane stride must be even)
TAP_PAIRS = [(0, 2), (3, 5), (6, 8), (1, 7)]
TAP_SINGLE = 4
TAP_ORDER = [0, 2, 3, 5, 6, 8, 1, 7, 4]  # host packs taps in this order
WS = 64.0           # fp8 weight scale (power of two)


def _toff(t):
    return (t // 3) * WP + (t % 3)


def _ntiles(total):
    """Balanced tile sizes <= NT (avoids tiny runt tiles, which are
    LDWEIGHTS-bound on hardware when DoubleRow disables fast weight load)."""
    n = -(-total // NT)
    base, rem = divmod(total, n)
    out = []
    p = 0
    for i in range(n):
        sz = base + (1 if i < rem else 0)
        out.append((p, sz))
        p += sz
    return out


def _legalize_waits(nc):
    """Split multi-semaphore waits onto sequencer NOPs.

    This build's walrus codegen accepts at most one sync wait per engine
    instruction; the tile scheduler can attach several (cross-engine RAW +
    buffer-recycle WAW).  NOP is sequencer-only and may carry a wait, so we
    peel all but the last wait onto NOPs placed immediately before the
    instruction on the same engine.
    """
    def mk_nop(engine, wait):
        bi = nc.engines[engine].nop(nofuse=True)
        mi = bi.ins
        host_bb = nc.cur_bb.bb
        lst = list(host_bb.instructions)
        assert lst and lst[-1].name == mi.name
        host_bb.instructions = lst[:-1]
        mi.sync_info = mybir.SyncInfo(on_wait=[wait], on_update=[])
        return mi

    for fn in nc.m.functions:
        for bb in fn.blocks:
            insts = list(bb.instructions)
            out = []
            changed = False
            for ins in insts:
                si = ins.sync_info
                if si is not None and si.on_wait and len(si.on_wait) > 1:
                    waits = list(si.on_wait)
                    for w in waits[:-1]:
                        out.append(mk_nop(ins.engine, w))
                    ins.sync_info = mybir.SyncInfo(
                        on_wait=[waits[-1]], on_update=list(si.on_update or []))
                    changed = True
                out.append(ins)
            if changed:
                bb.instructions = out


def _dr_rhs(tile_ap, base_off, delta, nn):
    """[P, 2, nn] overlapping view: plane j at base_off + j*delta."""
    return bass.AP(tensor=tile_ap.tensor, offset=tile_ap.offset + base_off,
                   ap=[list(tile_ap.ap[0]), [delta, 2], [1, nn]])


def _conv9(nc, psum_pool, w9, x_ap, in_base, nn, outP):
    """Emit folded 3x3 conv matmuls for one output tile; returns psum tile.

    w9: [P, 9, outP] fp8 weight AP in TAP_ORDER pair layout.
    x_ap: flat [P, L] fp8 input AP; reads at in_base + tap offsets.
    """
    ps = psum_pool.tile([outP, NT], F32, tag="ps")
    for i, (a, b) in enumerate(TAP_PAIRS):
        oa, ob = _toff(a), _toff(b)
        rhs = _dr_rhs(x_ap, in_base + oa, ob - oa, nn)
        nc.tensor.matmul(ps[:, :nn], w9[:, 2 * i:2 * i + 2, :], rhs,
                         start=(i == 0), stop=False, perf_mode=DR)
    off4 = _toff(TAP_SINGLE)
    nc.tensor.matmul(ps[:, :nn], w9[:, 8, :],
                     x_ap[:, in_base + off4:in_base + off4 + nn],
                     start=False, stop=True)
    return ps


def _conv9_chunk(nc, psum_pool, w9, x_ap, tiles, in_off, outP):
    """Folded 3x3 conv over several output tiles with each tap-pair's
    weights loaded once per chunk (hardware LDWEIGHTS amortization).
    Returns the list of psum tiles (one per (n0, nn) in tiles)."""
    pss = [psum_pool.tile([outP, NT], F32, tag="ps", name=f"psc{j}")
           for j in range(len(tiles))]
    for i, (a, b) in enumerate(TAP_PAIRS):
        oa, ob = _toff(a), _toff(b)
        w = w9[:, 2 * i:2 * i + 2, :]
        for ps, (n0, nn) in zip(pss, tiles):
            rhs = _dr_rhs(x_ap, in_off + n0 + oa, ob - oa, nn)
            nc.tensor.matmul(ps[:, :nn], w, rhs, start=(i == 0), stop=False,
                             perf_mode=DR, skip_group_check=True)
    off4 = _toff(TAP_SINGLE)
    for ps, (n0, nn) in zip(pss, tiles):
        base = in_off + n0 + off4
        nc.tensor.matmul(ps[:, :nn], w9[:, 8, :], x_ap[:, base:base + nn],
                         start=False, stop=True, skip_group_check=True)
    return pss


def _chunks(seq, k):
    return [seq[i:i + k] for i in range(0, len(seq), k)]


def build_graph():
    nc = bass.Bass()

    # ---- per-core inputs ----
    xres = nc.declare_dram_parameter("xres", [96, R_MID, WP], BF16, isOutput=False)
    xf8e = nc.declare_dram_parameter("xf8", [96, NPIX_IN + 2], F8, isOutput=False)
    w_qkvf = nc.declare_dram_parameter("w_qkvf", [96, 3, 9, 96], F8, isOutput=False)
    w_pEf = nc.declare_dram_parameter("w_pEf", [96, 3, 9, 128], F8, isOutput=False)
    w_pE12f = nc.declare_dram_parameter("w_pE12f", [96, 3, 9, 128], F8, isOutput=False)
    w_pof = nc.declare_dram_parameter("w_pof", [128, 2, 96], F8, isOutput=False)
    w_po2 = nc.declare_dram_parameter("w_po2", [128, 96], F8, isOutput=False)
    w_mid = nc.declare_dram_parameter("w_mid", [96, 96], BF16, isOutput=False)
    ident = nc.declare_dram_parameter("ident", [96, 96], BF16, isOutput=False)
    mneg = nc.declare_dram_parameter("mneg", [48, 48], F32, isOutput=False)
    t12e = nc.declare_dram_parameter("t12e", [48, 2], F32, isOutput=False)
    masks = nc.declare_dram_parameter("masks", [96, 2], F32, isOutput=False)
    out_ext = nc.declare_dram_parameter("out", [96, R_OUT, W], BF16, isOutput=True)

    stats_in = nc.dram_tensor("stats_in", [48, 112], BF16)
    stats_ag = nc.dram_tensor("stats_ag", [4, 48, 112], BF16)

    with tile.TileContext(nc) as tc, ExitStack() as ctx:
        wpool = ctx.enter_context(tc.tile_pool(name="weights", bufs=1))
        psum = ctx.enter_context(tc.tile_pool(name="psum", bufs=6, space="PSUM"))
        small = ctx.enter_context(tc.tile_pool(name="small", bufs=1))
        _x12_cm = tc.tile_pool(name="pool_x12", bufs=1)
        pool_x12 = _x12_cm.__enter__()
        x12p = pool_x12.tile([96, NPIX_MID], BF16, tag="x12p")
        x12f8 = pool_x12.tile([96, NPIX_MID + 2], F8, tag="x12f8")
        _xf_cm = tc.tile_pool(name="pool_xf", bufs=1)
        bigp = _xf_cm.__enter__()

        # ---- load weights (w_qkvf + xf8 first: they gate the first conv) ----
        w_qkvf_t = wpool.tile([96, 3, 9, 96], F8, tag="w_qkvf")
        w_pEf_t = wpool.tile([96, 3, 9, 128], F8, tag="w_pEf")
        w_pE12f_t = wpool.tile([96, 3, 9, 128], F8, tag="w_pE12f")
        w_pof_t = wpool.tile([128, 2, 96], F8, tag="w_pof")
        w_po2_t = wpool.tile([128, 96], F8, tag="w_po2")
        w_mid_t = wpool.tile([96, 96], BF16, tag="w_mid")
        ident_t = wpool.tile([96, 96], BF16, tag="ident")
        mneg_t = wpool.tile([48, 48], F32, tag="mneg")
        t12_t = wpool.tile([48, 2], F32, tag="t12")
        masks_t = wpool.tile([96, 2], F32, tag="masks")
        xf8 = bigp.tile([96, NPIX_IN + 2], F8, tag="xf8")
        nc.sync.dma_start(w_qkvf_t[:], w_qkvf[:])
        # split across chunks/queues so the first conv stripe starts sooner
        X0, XA, XB = 7 * WP, 19 * WP, 35 * WP
        nc.sync.dma_start(xf8[:, 0:X0], xf8e[:, 0:X0])
        nc.sync.dma_start(xf8[:, X0:XA], xf8e[:, X0:XA])
        nc.sync.dma_start(xf8[:, XA:XB], xf8e[:, XA:XB])
        nc.gpsimd.dma_start(xf8[:, XB:], xf8e[:, XB:])
        for dst, src in [(ident_t, ident), (w_pEf_t, w_pEf),
                         (w_pE12f_t, w_pE12f), (w_pof_t, w_pof),
                         (w_po2_t, w_po2), (w_mid_t, w_mid),
                         (mneg_t, mneg), (t12_t, t12e), (masks_t, masks)]:
            nc.sync.dma_start(dst[:], src[:])

        # ---- semaphore warmups: each engine picks up one new DMA-queue
        # semaphore per instruction (codegen allows a single sync wait).
        # Only the tensors gating the first conv stripe warm up front; the
        # rest warm after stripe 0 so the PE does not stall on their DMAs.
        _warm_cm = tc.tile_pool(name="warm_ps", bufs=1, space="PSUM")
        wu = _warm_cm.__enter__().tile([2, 16], F32, tag="wu")
        for wi, wview in enumerate([w_qkvf_t[0:2, 0, 0, 0:2], xf8[0:2, 0:2]]):
            nc.tensor.matmul(wu[:, 2 * wi:2 * wi + 2], wview, wview,
                             start=True, stop=True)
        wub = small.tile([2, 10], BF16, tag="wub")
        nc.vector.tensor_copy(wub[:, 2:4], mneg_t[0:2, 0:2])
        nc.vector.tensor_copy(wub[:, 4:6], t12_t[0:2, 0:2])
        nc.vector.tensor_copy(wub[:, 6:8], masks_t[0:2, 0:2])

        # ---- stage B+D: q/k folded convs interleaved with gram + sumsq ----
        # (per 16-row stripe, so the AllReduce can start right after the
        # last stripe instead of after a separate full gram pass)
        _v_cm = tc.tile_pool(name="pool_v", bufs=1)
        pool_v = _v_cm.__enter__()
        _qk_cm = tc.tile_pool(name="pool_qk", bufs=1)
        pool_qk = _qk_cm.__enter__()
        qt = pool_qk.tile([96, QK_LEN], BF16, tag="qt")
        kt = pool_qk.tile([96, QK_LEN], BF16, tag="kt")
        vt = pool_v.tile([96, NPIX_MID], BF16, tag="vt")
        xf8v = xf8[:]
        qv3 = qt[:].rearrange("c (h w) -> c h w", w=WP)
        kv3 = kt[:].rearrange("c (h w) -> c h w", w=WP)
        ssqk = small.tile([96, 8], F32, tag="ssqk")
        statspack = small.tile([48, 112], BF16, tag="statspack")
        GB = 16
        _junk_cm = tc.tile_pool(name="junk", bufs=1)
        junk = _junk_cm.__enter__().tile([96, GB * W], BF16, tag="junk")
        jv = junk[:].rearrange("c (h w) -> c h w", w=W)
        _gsb_cm = tc.tile_pool(name="gram_sb", bufs=3)
        gsb = _gsb_cm.__enter__()
        _gacc_cm = tc.tile_pool(name="gram_acc", bufs=1, space="PSUM")
        G12t_ps = _gacc_cm.__enter__().tile([48, 96], F32, tag="G12t")
        nstep = 0
        NSTEPS = 32
        for stripe in range(4):
            rb0 = GB * stripe
            # q, k convs for rows rb0..rb0+15 (window rows +1)
            for g, dst in [(0, qt), (1, kt)]:
                w9 = w_qkvf_t[:, g]
                for tl in _chunks(_ntiles(GB * WP), 2):
                    pss = _conv9_chunk(nc, psum, w9, xf8v, tl,
                                       WP + rb0 * WP, 96)
                    for ps, (n0, nn) in zip(pss, tl):
                        nc.vector.tensor_copy(
                            dst[:, rb0 * WP + n0:rb0 * WP + n0 + nn],
                            ps[:, :nn])
            if stripe == 0:
                # late warmups: these DMAs have landed by now
                for wi, wview in enumerate([
                        ident_t[0:2, 0:2], w_pEf_t[0:2, 0, 0, 0:2],
                        w_pE12f_t[0:2, 0, 0, 0:2], w_pof_t[0:2, 0, 0:2],
                        w_po2_t[0:2, 0:2], w_mid_t[0:2, 0:2]]):
                    nc.tensor.matmul(wu[:, 2 * wi + 4:2 * wi + 6],
                                     wview, wview, start=True, stop=True)
            # sumsq of this stripe (interior cols only)
            nc.scalar.activation(jv[:], qv3[:, rb0:rb0 + GB, 1:1 + W],
                                 AF.Square,
                                 accum_out=ssqk[:, stripe:stripe + 1])
            nc.scalar.activation(jv[:], kv3[:, rb0:rb0 + GB, 1:1 + W],
                                 AF.Square,
                                 accum_out=ssqk[:, 4 + stripe:5 + stripe])
            # gram: 2 rows per step (8 transposes, 1 evac, 8 matmuls)
            for r in range(rb0, rb0 + GB, 2):
                tp = psum.tile([128, 768], BF16, tag="ps")
                for j in range(4):
                    rr, half = r + j // 2, j % 2
                    c0 = 1 + 128 * half
                    o = 192 * j
                    nc.tensor.transpose(tp[:, o:o + 96],
                                        qv3[:, rr, c0:c0 + 128], ident_t[:])
                    nc.tensor.transpose(tp[:, o + 96:o + 192],
                                        kv3[:, rr, c0:c0 + 128], ident_t[:])
                ts = gsb.tile([128, 768], BF16, tag="ts")
                nc.vector.tensor_copy(ts[:], tp[:])
                for j in range(4):
                    o = 192 * j
                    st = (nstep == 0 and j == 0)
                    sp = (nstep == NSTEPS - 1 and j == 3)
                    nc.tensor.matmul(G12t_ps[:, 0:48], ts[:, o + 96:o + 144],
                                     ts[:, o + 48:o + 96], start=st, stop=sp,
                                     skip_group_check=True)
                    nc.tensor.matmul(G12t_ps[:, 48:96], ts[:, o + 144:o + 192],
                                     ts[:, o:o + 48], start=st, stop=sp,
                                     skip_group_check=True)
                nstep += 1
        nc.vector.tensor_copy(statspack[:, 0:96], G12t_ps[:])
        _gacc_cm.__exit__(None, None, None)
        _gsb_cm.__exit__(None, None, None)
        _junk_cm.__exit__(None, None, None)
        _warm_cm.__exit__(None, None, None)
        nc.vector.tensor_copy(statspack[:, 96:104], ssqk[0:48, :])
        nc.gpsimd.dma_start(statspack[:, 104:112], ssqk[48:96, :])
        _qk_cm.__exit__(None, None, None)

        # ---- stage E: single AllReduce over the 4 cores sharing a batch ----
        # stats [48, 112]: 0:48 gram1^T, 48:96 gram2^T, then 4-stripe partial
        # sumsq each for q1 (96:100), k1 (100:104), q2 (104:108), k2 (108:112)
        # AllGather + 3 local adds: ~2x cheaper than AllReduce on the wire
        nc.sync.dma_start(stats_in[:], statspack[:])
        nc.gpsimd.collective_compute(
            "AllGather", ALU.bypass,
            ins=[stats_in[:]], outs=[stats_ag[:]],
            replica_groups=[[0, 1, 2, 3], [4, 5, 6, 7]],
        )
        statsRg = small.tile([48, 4, 112], BF16, tag="statsRg")
        nc.sync.dma_start(statsRg[:], stats_ag[:].transpose([1, 0, 2]))
        statsR = small.tile([48, 112], F32, tag="statsR")
        sR01 = small.tile([48, 112], F32, tag="sR01")
        nc.vector.tensor_tensor(sR01[:], statsRg[:, 0], statsRg[:, 1], ALU.add)
        nc.vector.tensor_tensor(statsR[:], statsRg[:, 2], statsRg[:, 3], ALU.add)
        nc.vector.tensor_tensor(statsR[:], statsR[:], sR01[:], ALU.add)

        # residual window lands in the space qt/kt vacated
        _xb_cm = tc.tile_pool(name="pool_xb", bufs=1)
        xbp = _xb_cm.__enter__()
        xb = xbp.tile([96, NPIX_MID], BF16, tag="xb")
        xrf = xres[:].rearrange("c h w -> c (h w)")
        # chunked so the small stats DMA isn't stuck behind one 9us transfer
        XR = 11 * WP
        for c0 in range(0, NPIX_MID, XR):
            c1 = min(NPIX_MID, c0 + XR)
            nc.sync.dma_start(xb[:, c0:c1], xrf[:, c0:c1])
        nc.vector.tensor_copy(wub[:, 0:2], xb[0:2, 0:2])

        # ---- v conv emitted after the collective dispatch so PE overlaps it
        wv9 = w_qkvf_t[:, 2]
        for tl in _chunks(_ntiles(NPIX_MID), 2):
            pss = _conv9_chunk(nc, psum, wv9, xf8v, tl, 0, 96)
            for ps, (n0, nn) in zip(pss, tl):
                nc.vector.tensor_copy(vt[:, n0:n0 + nn], ps[:, :nn])
        # zero padded columns of v (q/k handled by AP exclusion)
        nc.vector.memset(vt[:].rearrange("c (h w) -> c h w", w=WP)[:, :, 0:1], 0.0)
        nc.vector.memset(vt[:].rearrange("c (h w) -> c h w", w=WP)[:, :, WP - 1:WP], 0.0)

        # ---- stage F: normalize, softmax, fused (mid @ A) ----
        # q/k carry the x64 fp8 weight scale; it cancels exactly in
        # gram * rsq_q * rsq_k.  v's scale is folded into w_mid (host /64).
        ssf = small.tile([48, 4], F32, tag="ssf")
        for i in range(4):
            nc.vector.reduce_sum(ssf[:, i:i + 1],
                                 statsR[:, 96 + 4 * i:100 + 4 * i],
                                 axis=mybir.AxisListType.X)
        rsq = small.tile([48, 4], F32, tag="rsq")
        nc.scalar.activation(rsq[:], ssf[:], AF.Sqrt)
        nc.vector.reciprocal(rsq[:], rsq[:])
        A_bd = small.tile([96, 96], BF16, tag="A_bd")
        nc.vector.memset(A_bd[:], 0.0)
        id48 = ident_t[0:48, 0:48]
        for gi in range(2):
            Bt = statsR[:, 0:48] if gi == 0 else statsR[:, 48:96]
            kcol, qcol, tcol = (1, 2, 0) if gi == 0 else (3, 0, 1)
            Bs = small.tile([48, 48], BF16, tag=f"Bs{gi}")
            nc.vector.tensor_scalar_mul(Bs[:], Bt, rsq[:, kcol:kcol + 1])
            ps_t = psum.tile([48, 48], BF16, tag="ps")
            nc.tensor.transpose(ps_t[:], Bs[:], id48)
            sc = small.tile([48, 1], F32, tag=f"sc{gi}")
            nc.vector.tensor_tensor(sc[:], rsq[:, qcol:qcol + 1],
                                    t12_t[:, tcol:tcol + 1], ALU.mult)
            Gw = small.tile([48, 48], F32, tag=f"Gw{gi}")
            nc.vector.scalar_tensor_tensor(Gw[:], ps_t[:], sc[:, 0:1],
                                           mneg_t[:], ALU.mult, ALU.add)
            # no max-subtraction: logits are cosine-sims * t (|t|~1), and the
            # -1e30 mask exponentiates to exactly 0 -- exp is overflow-safe
            E = small.tile([48, 48], F32, tag=f"E{gi}")
            nc.scalar.activation(E[:], Gw[:], AF.Exp)
            sm = small.tile([48, 1], F32, tag=f"sm{gi}")
            nc.vector.reduce_sum(sm[:], E[:], axis=mybir.AxisListType.X)
            nc.vector.reciprocal(sm[:], sm[:])
            A = small.tile([48, 48], BF16, tag=f"A{gi}")
            nc.vector.tensor_scalar_mul(A[:], E[:], sm[:, 0:1])
            if gi == 0:
                nc.vector.tensor_copy(A_bd[0:48, 0:48], A[:])
            else:
                nc.sync.dma_start(A_bd[48:96, 48:96], A[:])
        # F = A_bd^T @ w_mid  (so F^T = W_mid @ A: attention + mid in one mm)
        F_ps = psum.tile([96, 96], F32, tag="ps")
        nc.tensor.matmul(F_ps[:], A_bd[:], w_mid_t[:], start=True, stop=True)
        F_sb = small.tile([96, 96], BF16, tag="F_sb")
        nc.vector.tensor_copy(F_sb[:], F_ps[:])

        # ---- stage G: fused attn-apply + mid conv + residual -> x' ----
        # fp8 casts (ACT) are inlined per tile so stage H starts immediately;
        # edge-mask rows 0 / 65 gate only the first / last two casts.
        nc.vector.memset(x12f8[:, NPIX_MID:NPIX_MID + 2], 0.0)
        tiles = _ntiles(NPIX_MID)
        ncast = len(tiles)
        # residual folded into PSUM via identity matmul: the evacuation
        # becomes a pure copy (ACT) and the fp8 cast rides GPSIMD, so the
        # phase isn't paced by DVE psum-reads.  Chunked so F / identity
        # weights load once per chunk, not per tile.
        for tl in _chunks(list(enumerate(tiles)), 2):
            pss = [psum.tile([96, NT], F32, tag="ps", name=f"psc{j}")
                   for j in range(len(tl))]
            for ps, (i, (n0, nn)) in zip(pss, tl):
                nc.tensor.matmul(ps[:, :nn], F_sb[:], vt[:, n0:n0 + nn],
                                 start=True, stop=False,
                                 skip_group_check=True)
            for ps, (i, (n0, nn)) in zip(pss, tl):
                nc.tensor.matmul(ps[:, :nn], ident_t[:], xb[:, n0:n0 + nn],
                                 start=False, stop=True,
                                 skip_group_check=True)
            for ps, (i, (n0, nn)) in zip(pss, tl):
                nc.scalar.copy(x12p[:, n0:n0 + nn], ps[:, :nn])
                if i == 0:
                    nc.vector.tensor_scalar_mul(x12p[:, 0:WP], x12p[:, 0:WP],
                                                masks_t[:, 0:1])
                if i < ncast - 2:
                    nc.gpsimd.tensor_copy(x12f8[:, n0:n0 + nn],
                                          x12p[:, n0:n0 + nn])
        nc.vector.tensor_scalar_mul(x12p[:, (R_MID - 1) * WP:],
                                    x12p[:, (R_MID - 1) * WP:], masks_t[:, 1:2])
        for n0, nn in tiles[-2:]:
            nc.gpsimd.tensor_copy(x12f8[:, n0:n0 + nn], x12p[:, n0:n0 + nn])
        _xb_cm.__exit__(None, None, None)
        _v_cm.__exit__(None, None, None)
        _xf_cm.__exit__(None, None, None)

        # ---- stage H: folded stage-2 convs, gated products, output ----
        # po is emitted one block behind the convs so the PE never stalls on
        # the gg (DVE) product at block boundaries; the last blocks are
        # smaller to shorten the un-overlapped tail.
        BLOCKS = [(0, 16), (16, 16), (32, 16), (48, 8), (56, 8)]
        NBLK = len(BLOCKS)
        x12v = x12f8[:]
        with tc.tile_pool(name="blk", bufs=1) as bp:
            ggs = {}

            def convs_block(blk):
                r0, rows = BLOCKS[blk]
                ol = rows * WP
                in_off = r0 * WP
                gg = bp.tile([128, 3, 16 * WP], F8, tag=f"gg{blk % 2}")
                ggs[blk] = gg
                for m in range(3):
                    wE9 = w_pEf_t[:, m]
                    w129 = w_pE12f_t[:, m]
                    g_m = bp.tile([128, 16 * WP], BF16, tag=f"g{m}")
                    for tl in _chunks(_ntiles(ol), 2):
                        pss = _conv9_chunk(nc, psum, wE9, x12v, tl,
                                           in_off, 128)
                        for ps, (n0, nn) in zip(pss, tl):
                            nc.scalar.activation(g_m[:, n0:n0 + nn],
                                                 ps[:, :nn], AF.Gelu,
                                                 scale=1.0 / WS)
                    if blk < 3:
                        for tl in _chunks(_ntiles(ol), 2):
                            pss = _conv9_chunk(nc, psum, w129, x12v, tl,
                                               in_off, 128)
                            # fused: gg = 8/WS * psum_e * gelu_g from PSUM
                            # (x8 so the fp8 gg feeds a DoubleRow po)
                            for ps, (n0, nn) in zip(pss, tl):
                                nc.vector.scalar_tensor_tensor(
                                    gg[:, m, n0:n0 + nn], ps[:, :nn],
                                    8.0 / WS, g_m[:, n0:n0 + nn],
                                    ALU.mult, ALU.mult)
                    else:
                        # tail blocks: keep DVE free for the outf adds --
                        # evacuate on ACT, multiply on GPSIMD (SBUF-only)
                        e_m = bp.tile([128, 8 * WP], BF16, tag="e_t")
                        for tl in _chunks(_ntiles(ol), 2):
                            pss = _conv9_chunk(nc, psum, w129, x12v, tl,
                                               in_off, 128)
                            for ps, (n0, nn) in zip(pss, tl):
                                nc.scalar.activation(e_m[:, n0:n0 + nn],
                                                     ps[:, :nn], AF.Copy,
                                                     scale=8.0 / WS)
                                nc.gpsimd.tensor_tensor(
                                    gg[:, m, n0:n0 + nn], e_m[:, n0:n0 + nn],
                                    g_m[:, n0:n0 + nn], ALU.mult)

            def po_block(blk):
                r0, rows = BLOCKS[blk]
                ol = rows * WP
                out_off = (r0 + 1) * WP
                gg = ggs.pop(blk)
                ggv = gg[:].rearrange("c m n -> c (m n)")
                GOL = 16 * WP
                outf = bp.tile([96, 16 * WP], BF16, tag=f"outf{blk % 2}")
                for tl in _chunks(_ntiles(ol), 2):
                    pss = [psum.tile([96, NT], F32, tag="ps", name=f"psc{j}")
                   for j in range(len(tl))]
                    for ps, (n0, nn) in zip(pss, tl):
                        rhs = _dr_rhs(ggv, n0, GOL, nn)
                        nc.tensor.matmul(ps[:, :nn], w_pof_t[:], rhs,
                                         start=True, stop=False, perf_mode=DR,
                                         skip_group_check=True)
                    for ps, (n0, nn) in zip(pss, tl):
                        nc.tensor.matmul(
                            ps[:, :nn], w_po2_t[:],
                            ggv[:, 2 * GOL + n0:2 * GOL + n0 + nn],
                            start=False, stop=True, skip_group_check=True)
                    # po weights carry xWS, gg carries x8 -> descale fused
                    for ps, (n0, nn) in zip(pss, tl):
                        nc.vector.scalar_tensor_tensor(
                            outf[:, n0:n0 + nn], ps[:, :nn], 1.0 / (8.0 * WS),
                            x12p[:, out_off + n0:out_off + n0 + nn],
                            ALU.mult, ALU.add)
                ov = outf[:].rearrange("c (h w) -> c h w", w=WP)
                nc.sync.dma_start(out_ext[:, r0:r0 + rows, :],
                                  ov[:, 0:rows, 1:1 + W])

            convs_block(0)
            for blk in range(1, NBLK):
                convs_block(blk)
                po_block(blk - 1)
            po_block(NBLK - 1)
        _x12_cm.__exit__(None, None, None)
    _legalize_waits(nc)
    return nc


_NC_CACHE = None


def _get_nc():
    global _NC_CACHE
    if _NC_CACHE is None:
        _NC_CACHE = build_graph()
    return _NC_CACHE


def _bf16(a):
    return np.ascontiguousarray(a.astype(ml_dtypes.bfloat16))


def _f8(a):
    return np.ascontiguousarray(
        np.clip(a, -240.0, 240.0).astype(ml_dtypes.float8_e4m3))


def _prep_weights(ins):
    dim = DIM
    out = {}
    # folded qkv: per group g and tap t, M_t = (diag(d_t) @ W)^T = W^T * d[cols]
    wq = np.zeros((3, 9, 96, 96), np.float32)
    for g in range(3):
        W1 = ins["qkv1_w"][g * dim:(g + 1) * dim, :, 0, 0]  # [48 out, 48 in]
        W2 = ins["qkv2_w"][g * dim:(g + 1) * dim, :, 0, 0]
        for pos, t in enumerate(TAP_ORDER):
            d1 = ins["qkv1_dw"][g * dim:(g + 1) * dim, 0, t // 3, t % 3]
            d2 = ins["qkv2_dw"][g * dim:(g + 1) * dim, 0, t // 3, t % 3]
            wq[g, pos, 0:48, 0:48] = W1.T * d1[None, :]
            wq[g, pos, 48:96, 48:96] = W2.T * d2[None, :]
    out["w_qkvf"] = _f8(np.transpose(wq, (2, 0, 1, 3)) * WS)

    Wp = ins["pE_w"][:, :, 0, 0]          # [384 out, 96 in]
    dpe = ins["pE_dw"][:, 0]              # [384, 3, 3]
    wpe = np.zeros((3, 9, 96, 128), np.float32)
    for pos, t in enumerate(TAP_ORDER):
        M = Wp.T * dpe[:, t // 3, t % 3][None, :]   # [96, 384]
        for m in range(3):
            wpe[m, pos] = M[:, 128 * m:128 * (m + 1)]
    out["w_pEf"] = _f8(np.transpose(wpe, (2, 0, 1, 3)) * WS)

    W12 = np.zeros((96, 384), np.float32)
    W12[0:48, 0:192] = ins["pE1_w"][:, :, 0, 0].T
    W12[48:96, 192:384] = ins["pE2_w"][:, :, 0, 0].T
    d12 = np.concatenate([ins["pE1_dw"][:, 0], ins["pE2_dw"][:, 0]], axis=0)
    w12 = np.zeros((3, 9, 96, 128), np.float32)
    for pos, t in enumerate(TAP_ORDER):
        M = W12 * d12[:, t // 3, t % 3][None, :]
        for m in range(3):
            w12[m, pos] = M[:, 128 * m:128 * (m + 1)]
    out["w_pE12f"] = _f8(np.transpose(w12, (2, 0, 1, 3)) * WS)

    wpo = np.concatenate([ins["po1_w"][:, :, 0, 0].T,
                          ins["po2_w"][:, :, 0, 0].T], axis=0)  # [384, 96]
    out["w_pof"] = _f8(np.stack([wpo[0:128], wpo[128:256]], axis=1) * WS)
    out["w_po2"] = _f8(wpo[256:384] * WS)

    wm = np.zeros((96, 96), np.float32)
    wm[0:48, 0:48] = ins["mid1_w"][:, :, 0, 0].T
    wm[48:96, 48:96] = ins["mid2_w"][:, :, 0, 0].T
    out["w_mid"] = _bf16(wm / WS)   # absorbs v's fp8 weight scale
    out["ident"] = _bf16(np.eye(96, dtype=np.float32))
    hb = np.repeat(np.arange(8), 6)
    out["mneg"] = np.where(hb[:, None] == hb[None, :], 0.0, -1e30).astype(np.float32)
    t12 = np.zeros((48, 2), np.float32)
    t12[:, 0] = np.repeat(ins["t1"][:, 0, 0], 6)
    t12[:, 1] = np.repeat(ins["t2"][:, 0, 0], 6)
    out["t12e"] = t12
    return out


def _shard_x(x1, x2, si):
    """x1/x2: [48, 256, 256] one batch -> (xres bf16 [96,66,258],
    xf8 fp8 [96, 68*258+2]) shards, zero padded."""
    shf = np.zeros((96, R_IN, WP), np.float32)
    r0 = 64 * si - 2
    lo, hi = max(0, r0), min(H, r0 + R_IN)
    shf[0:48, lo - r0:hi - r0, 1:1 + W] = x1[:, lo:hi, :]
    shf[48:96, lo - r0:hi - r0, 1:1 + W] = x2[:, lo:hi, :]
    xres = shf[:, 1:1 + R_MID].astype(ml_dtypes.bfloat16)
    xf8 = np.zeros((96, NPIX_IN + 2), np.float32)
    xf8[:, 0:NPIX_IN] = shf.reshape(96, NPIX_IN)
    return np.ascontiguousarray(xres), _f8(xf8)


LAST_EXEC_NS = None


def _kernel_device(**inputs):
    global LAST_EXEC_NS
    nc = _get_nc()
    wts = _prep_weights(inputs)
    x1 = np.asarray(inputs["x1"], np.float32)
    x2 = np.asarray(inputs["x2"], np.float32)
    in_maps = []
    for core in range(8):
        bi, si = core // 4, core % 4
        m = dict(wts)
        xres, xf8 = _shard_x(x1[bi], x2[bi], si)
        m["xres"] = xres
        m["xf8"] = xf8
        mk = np.ones((96, 2), np.float32)
        if si == 0:
            mk[:, 0] = 0.0
        if si == 3:
            mk[:, 1] = 0.0
        m["masks"] = mk
        in_maps.append(m)
    trace = bool(os.environ.get("KERNEL_TRACE"))
    res = run_bass_kernel_spmd(nc, in_maps, core_ids=list(range(8)),
                               trace=trace)
    if trace:
        LAST_EXEC_NS = res.exec_time_ns
    out = np.zeros((B, 2 * DIM, H, W), np.float32)
    for core in range(8):
        bi, si = core // 4, core % 4
        out[bi, :, 64 * si:64 * si + 64, :] = \
            np.asarray(res.results[core]["out"]).astype(np.float32)
    return out


if __name__ == "__main__":
    pass


# ---------- host fallback (pure numpy, exact) ----------
def _erf(x):
    # Abramowitz-Stegun 7.1.26, max abs err ~1.5e-7
    sgn = np.sign(x)
    ax = np.abs(x)
    t = 1.0 / (1.0 + 0.3275911 * ax)
    y = 1.0 - (((((1.061405429 * t - 1.453152027) * t) + 1.421413741) * t
                - 0.284496736) * t + 0.254829592) * t * np.exp(-ax * ax)
    return sgn * y


def _pw(x, w):
    return np.einsum("oc,bchw->bohw", w[:, :, 0, 0], x, optimize=True)


def _dw3(x, w):
    b, C, Hh, Ww = x.shape
    xp = np.zeros((b, C, Hh + 2, Ww + 2), x.dtype)
    xp[:, :, 1:-1, 1:-1] = x
    out = np.zeros_like(x)
    for di in range(3):
        for dj in range(3):
            out += w[None, :, 0, di, dj, None, None] * xp[:, :, di:di + Hh, dj:dj + Ww]
    return out


def _l2n(t):
    n = np.sqrt((t * t).sum(axis=-1, keepdims=True))
    return t / np.maximum(n, 1e-12)


def _gelu(x):
    return 0.5 * x * (1.0 + _erf(x / np.sqrt(2.0).astype(np.float32)))


def _kernel_host(x1, x2, t1, t2, qkv1_w, qkv1_dw, qkv2_w, qkv2_dw,
                 mid1_w, mid2_w, pE_w, pE_dw, pE1_w, pE1_dw,
                 pE2_w, pE2_dw, po1_w, po2_w):
    b, c, h, w = x1.shape
    heads = t1.shape[0]
    ch = c // heads

    def to_heads(t):
        return t.reshape(b, heads, ch, h * w)

    qkv1 = _dw3(_pw(x1, qkv1_w), qkv1_dw)
    q1, k1, v1 = np.split(qkv1, 3, axis=1)
    qkv2 = _dw3(_pw(x2, qkv2_w), qkv2_dw)
    q2, k2, v2 = np.split(qkv2, 3, axis=1)
    q1, k1, v1 = to_heads(q1), to_heads(k1), to_heads(v1)
    q2, k2, v2 = to_heads(q2), to_heads(k2), to_heads(v2)
    q1, k1 = _l2n(q1), _l2n(k1)
    q2, k2 = _l2n(q2), _l2n(k2)
    attn1 = np.einsum("bhcn,bhdn->bhcd", q2, k1, optimize=True) * t1[None]
    attn2 = np.einsum("bhcn,bhdn->bhcd", q1, k2, optimize=True) * t2[None]

    def sm(a):
        a = a - a.max(-1, keepdims=True)
        e = np.exp(a)
        return e / e.sum(-1, keepdims=True)

    out1 = np.einsum("bhcd,bhdn->bhcn", sm(attn1), v1, optimize=True).reshape(b, c, h, w)
    out2 = np.einsum("bhcd,bhdn->bhcn", sm(attn2), v2, optimize=True).reshape(b, c, h, w)
    x1 = x1 + _pw(out1, mid1_w)
    x2 = x2 + _pw(out2, mid2_w)
    out = np.concatenate([x1, x2], axis=1)
    g = _dw3(_pw(out, pE_w), pE_dw)
    g1, g2 = np.split(g, 2, axis=1)
    y1 = _gelu(g1) * _dw3(_pw(x1, pE1_w), pE1_dw)
    y2 = _gelu(g2) * _dw3(_pw(x2, pE2_w), pE2_dw)
    return (out + _pw(y1, po1_w) + _pw(y2, po2_w)).astype(np.float32)


def kernel(**inputs):
    inputs = {k: np.asarray(v, np.float32) for k, v in inputs.items()}
    if _HAVE_BASS and not os.environ.get("KERNEL_FORCE_HOST"):
        try:
            return _kernel_device(**inputs)
        except Exception as e:
            sys.stderr.write(f"[kernel] device path failed ({type(e).__name__}: {e}); "
                             "falling back to host compute\n")
    return _kernel_host(**inputs)


# revision 71
# speedup vs baseline: 1.1044x; 1.1044x over previous
import sys, os
sys.path.insert(0, "/opt/trn_rl_repo")
import numpy as np
from contextlib import ExitStack

try:
    import ml_dtypes
    import concourse.bass as bass
    import concourse.mybir as mybir
    from concourse import tile
    from concourse.bass_utils import run_bass_kernel_spmd
    _HAVE_BASS = True
except Exception:
    _HAVE_BASS = False

BF16 = mybir.dt.bfloat16
F32 = mybir.dt.float32
F8 = mybir.dt.float8e4
AF = mybir.ActivationFunctionType
ALU = mybir.AluOpType
DR = mybir.MatmulPerfMode.DoubleRow

# geometry (hardcoded for this problem)
DIM = 48
HEADS = 8
CH = 6
B = 2
H = 256
W = 256
WP = W + 2          # padded width
R_IN = 68           # fp8 input rows per shard (64 owned + 2 halo each side)
R_MID = 66          # q/k/v, x' rows
R_OUT = 64          # owned output rows
NPIX_IN = R_IN * WP
NPIX_MID = R_MID * WP
QK_LEN = 64 * WP    # q/k computed only on rows 1..64 of the 66-row window
NT = 512            # matmul free-dim tile

# 3x3 tap pairing for fp8 DoubleRow (plane stride must be even)
TAP_PAIRS = [(0, 2), (3, 5), (6, 8), (1, 7)]
TAP_SINGLE = 4
TAP_ORDER = [0, 2, 3, 5, 6, 8, 1, 7, 4]  # host packs taps in this order
WS = 64.0           # fp8 weight scale (power of two)


def _toff(t):
    return (t // 3) * WP + (t % 3)


def _ntiles(total):
    """Balanced tile sizes <= NT (avoids tiny runt tiles, which are
    LDWEIGHTS-bound on hardware when DoubleRow disables fast weight load)."""
    n = -(-total // NT)
    base, rem = divmod(total, n)
    out = []
    p = 0
    for i in range(n):
        sz = base + (1 if i < rem else 0)
        out.append((p, sz))
        p += sz
    return out


def _legalize_waits(nc):
    """Split multi-semaphore waits onto sequencer NOPs.

    This build's walrus codegen accepts at most one sync wait per engine
    instruction; the tile scheduler can attach several (cross-engine RAW +
    buffer-recycle WAW).  NOP is sequencer-only and may carry a wait, so we
    peel all but the last wait onto NOPs placed immediately before the
    instruction on the same engine.
    """
    def mk_nop(engine, wait):
        bi = nc.engines[engine].nop(nofuse=True)
        mi = bi.ins
        host_bb = nc.cur_bb.bb
        lst = list(host_bb.instructions)
        assert lst and lst[-1].name == mi.name
        host_bb.instructions = lst[:-1]
        mi.sync_info = mybir.SyncInfo(on_wait=[wait], on_update=[])
        return mi

    for fn in nc.m.functions:
        for bb in fn.blocks:
            insts = list(bb.instructions)
            out = []
            changed = False
            for ins in insts:
                si = ins.sync_info
                if si is not None and si.on_wait and len(si.on_wait) > 1:
                    waits = list(si.on_wait)
                    for w in waits[:-1]:
                        out.append(mk_nop(ins.engine, w))
                    ins.sync_info = mybir.SyncInfo(
                        on_wait=[waits[-1]], on_update=list(si.on_update or []))
                    changed = True
                out.append(ins)
            if changed:
                bb.instructions = out


def _dr_rhs(tile_ap, base_off, delta, nn):
    """[P, 2, nn] overlapping view: plane j at base_off + j*delta."""
    return bass.AP(tensor=tile_ap.tensor, offset=tile_ap.offset + base_off,
                   ap=[list(tile_ap.ap[0]), [delta, 2], [1, nn]])


def _conv9(nc, psum_pool, w9, x_ap, in_base, nn, outP):
    """Emit folded 3x3 conv matmuls for one output tile; returns psum tile.

    w9: [P, 9, outP] fp8 weight AP in TAP_ORDER pair layout.
    x_ap: flat [P, L] fp8 input AP; reads at in_base + tap offsets.
    """
    ps = psum_pool.tile([outP, NT], F32, tag="ps")
    for i, (a, b) in enumerate(TAP_PAIRS):
        oa, ob = _toff(a), _toff(b)
        rhs = _dr_rhs(x_ap, in_base + oa, ob - oa, nn)
        nc.tensor.matmul(ps[:, :nn], w9[:, 2 * i:2 * i + 2, :], rhs,
                         start=(i == 0), stop=False, perf_mode=DR)
    off4 = _toff(TAP_SINGLE)
    nc.tensor.matmul(ps[:, :nn], w9[:, 8, :],
                     x_ap[:, in_base + off4:in_base + off4 + nn],
                     start=False, stop=True)
    return ps


def _conv9_chunk(nc, psum_pool, w9, x_ap, tiles, in_off, outP):
    """Folded 3x3 conv over several output tiles with each tap-pair's
    weights loaded once per chunk (hardware LDWEIGHTS amortization).
    Returns the list of psum tiles (one per (n0, nn) in tiles)."""
    pss = [psum_pool.tile([outP, NT], F32, tag="ps", name=f"psc{j}")
           for j in range(len(tiles))]
    for i, (a, b) in enumerate(TAP_PAIRS):
        oa, ob = _toff(a), _toff(b)
        w = w9[:, 2 * i:2 * i + 2, :]
        for ps, (n0, nn) in zip(pss, tiles):
            rhs = _dr_rhs(x_ap, in_off + n0 + oa, ob - oa, nn)
            nc.tensor.matmul(ps[:, :nn], w, rhs, start=(i == 0), stop=False,
                             perf_mode=DR, skip_group_check=True)
    off4 = _toff(TAP_SINGLE)
    for ps, (n0, nn) in zip(pss, tiles):
        base = in_off + n0 + off4
        nc.tensor.matmul(ps[:, :nn], w9[:, 8, :], x_ap[:, base:base + nn],
                         start=False, stop=True, skip_group_check=True)
    return pss


def _chunks(seq, k):
    return [seq[i:i + k] for i in range(0, len(seq), k)]


def build_graph():
    nc = bass.Bass()

    # ---- per-core inputs ----
    xres = nc.declare_dram_parameter("xres", [96, R_MID, WP], BF16, isOutput=False)
    xf8e = nc.declare_dram_parameter("xf8", [96, NPIX_IN + 2], F8, isOutput=False)
    w_qkvf = nc.declare_dram_parameter("w_qkvf", [96, 3, 9, 96], F8, isOutput=False)
    w_pEf = nc.declare_dram_parameter("w_pEf", [96, 3, 9, 128], F8, isOutput=False)
    w_pE12f = nc.declare_dram_parameter("w_pE12f", [96, 3, 9, 128], F8, isOutput=False)
    w_pof = nc.declare_dram_parameter("w_pof", [128, 2, 96], F8, isOutput=False)
    w_po2 = nc.declare_dram_parameter("w_po2", [128, 96], F8, isOutput=False)
    w_mid = nc.declare_dram_parameter("w_mid", [96, 96], BF16, isOutput=False)
    ident = nc.declare_dram_parameter("ident", [96, 96], BF16, isOutput=False)
    mneg = nc.declare_dram_parameter("mneg", [48, 48], F32, isOutput=False)
    t12e = nc.declare_dram_parameter("t12e", [48, 2], F32, isOutput=False)
    masks = nc.declare_dram_parameter("masks", [96, 2], F32, isOutput=False)
    out_ext = nc.declare_dram_parameter("out", [96, R_OUT, W], BF16, isOutput=True)

    stats_in = nc.dram_tensor("stats_in", [48, 112], BF16)
    stats_ag = nc.dram_tensor("stats_ag", [4, 48, 112], BF16)

    with tile.TileContext(nc) as tc, ExitStack() as ctx:
        wpool = ctx.enter_context(tc.tile_pool(name="weights", bufs=1))
        psum = ctx.enter_context(tc.tile_pool(name="psum", bufs=6, space="PSUM"))
        small = ctx.enter_context(tc.tile_pool(name="small", bufs=1))
        _x12_cm = tc.tile_pool(name="pool_x12", bufs=1)
        pool_x12 = _x12_cm.__enter__()
        x12p = pool_x12.tile([96, NPIX_MID], BF16, tag="x12p")
        x12f8 = pool_x12.tile([96, NPIX_MID + 2], F8, tag="x12f8")
        _xf_cm = tc.tile_pool(name="pool_xf", bufs=1)
        bigp = _xf_cm.__enter__()

        # ---- load weights (w_qkvf + xf8 first: they gate the first conv) ----
        w_qkvf_t = wpool.tile([96, 3, 9, 96], F8, tag="w_qkvf")
        w_pEf_t = wpool.tile([96, 3, 9, 128], F8, tag="w_pEf")
        w_pE12f_t = wpool.tile([96, 3, 9, 128], F8, tag="w_pE12f")
        w_pof_t = wpool.tile([128, 2, 96], F8, tag="w_pof")
        w_po2_t = wpool.tile([128, 96], F8, tag="w_po2")
        w_mid_t = wpool.tile([96, 96], BF16, tag="w_mid")
        ident_t = wpool.tile([96, 96], BF16, tag="ident")
        mneg_t = wpool.tile([48, 48], F32, tag="mneg")
        t12_t = wpool.tile([48, 2], F32, tag="t12")
        masks_t = wpool.tile([96, 2], F32, tag="masks")
        xf8 = bigp.tile([96, NPIX_IN + 2], F8, tag="xf8")
        nc.sync.dma_start(w_qkvf_t[:], w_qkvf[:])
        # split across chunks/queues so the first conv stripe starts sooner
        X0, XA, XB = 7 * WP, 19 * WP, 35 * WP
        nc.sync.dma_start(xf8[:, 0:X0], xf8e[:, 0:X0])
        nc.sync.dma_start(xf8[:, X0:XA], xf8e[:, X0:XA])
        nc.sync.dma_start(xf8[:, XA:XB], xf8e[:, XA:XB])
        nc.gpsimd.dma_start(xf8[:, XB:], xf8e[:, XB:])
        for dst, src in [(ident_t, ident), (w_pEf_t, w_pEf),
                         (w_pE12f_t, w_pE12f), (w_pof_t, w_pof),
                         (w_po2_t, w_po2), (w_mid_t, w_mid),
                         (mneg_t, mneg), (t12_t, t12e), (masks_t, masks)]:
            nc.sync.dma_start(dst[:], src[:])

        # ---- semaphore warmups: each engine picks up one new DMA-queue
        # semaphore per instruction (codegen allows a single sync wait).
        # Only the tensors gating the first conv stripe warm up front; the
        # rest warm after stripe 0 so the PE does not stall on their DMAs.
        _warm_cm = tc.tile_pool(name="warm_ps", bufs=1, space="PSUM")
        wu = _warm_cm.__enter__().tile([2, 16], F32, tag="wu")
        for wi, wview in enumerate([w_qkvf_t[0:2, 0, 0, 0:2], xf8[0:2, 0:2]]):
            nc.tensor.matmul(wu[:, 2 * wi:2 * wi + 2], wview, wview,
                             start=True, stop=True)
        wub = small.tile([2, 10], BF16, tag="wub")
        nc.vector.tensor_copy(wub[:, 2:4], mneg_t[0:2, 0:2])
        nc.vector.tensor_copy(wub[:, 4:6], t12_t[0:2, 0:2])
        nc.vector.tensor_copy(wub[:, 6:8], masks_t[0:2, 0:2])

        # ---- stage B+D: q/k folded convs interleaved with gram + sumsq ----
        # (per 16-row stripe, so the AllReduce can start right after the
        # last stripe instead of after a separate full gram pass)
        _v_cm = tc.tile_pool(name="pool_v", bufs=1)
        pool_v = _v_cm.__enter__()
        _qk_cm = tc.tile_pool(name="pool_qk", bufs=1)
        pool_qk = _qk_cm.__enter__()
        qt = pool_qk.tile([96, QK_LEN], BF16, tag="qt")
        kt = pool_qk.tile([96, QK_LEN], BF16, tag="kt")
        vt = pool_v.tile([96, NPIX_MID], BF16, tag="vt")
        xf8v = xf8[:]
        qv3 = qt[:].rearrange("c (h w) -> c h w", w=WP)
        kv3 = kt[:].rearrange("c (h w) -> c h w", w=WP)
        ssqk = small.tile([96, 8], F32, tag="ssqk")
        statspack = small.tile([48, 112], BF16, tag="statspack")
        GB = 16
        _junk_cm = tc.tile_pool(name="junk", bufs=1)
        junk = _junk_cm.__enter__().tile([96, GB * W], BF16, tag="junk")
        jv = junk[:].rearrange("c (h w) -> c h w", w=W)
        _gsb_cm = tc.tile_pool(name="gram_sb", bufs=3)
        gsb = _gsb_cm.__enter__()
        _gacc_cm = tc.tile_pool(name="gram_acc", bufs=1, space="PSUM")
        G12t_ps = _gacc_cm.__enter__().tile([48, 96], F32, tag="G12t")
        nstep = 0
        NSTEPS = 32
        for stripe in range(4):
            rb0 = GB * stripe
            # q, k convs for rows rb0..rb0+15 (window rows +1)
            for g, dst in [(0, qt), (1, kt)]:
                w9 = w_qkvf_t[:, g]
                for tl in _chunks(_ntiles(GB * WP), 2):
                    pss = _conv9_chunk(nc, psum, w9, xf8v, tl,
                                       WP + rb0 * WP, 96)
                    for ps, (n0, nn) in zip(pss, tl):
                        nc.vector.tensor_copy(
                            dst[:, rb0 * WP + n0:rb0 * WP + n0 + nn],
                            ps[:, :nn])
            if stripe == 0:
                # late warmups: these DMAs have landed by now
                for wi, wview in enumerate([
                        ident_t[0:2, 0:2], w_pEf_t[0:2, 0, 0, 0:2],
                        w_pE12f_t[0:2, 0, 0, 0:2], w_pof_t[0:2, 0, 0:2],
                        w_po2_t[0:2, 0:2], w_mid_t[0:2, 0:2]]):
                    nc.tensor.matmul(wu[:, 2 * wi + 4:2 * wi + 6],
                                     wview, wview, start=True, stop=True)
            # sumsq of this stripe (interior cols only)
            nc.scalar.activation(jv[:], qv3[:, rb0:rb0 + GB, 1:1 + W],
                                 AF.Square,
                                 accum_out=ssqk[:, stripe:stripe + 1])
            nc.scalar.activation(jv[:], kv3[:, rb0:rb0 + GB, 1:1 + W],
                                 AF.Square,
                                 accum_out=ssqk[:, 4 + stripe:5 + stripe])
            # gram: 2 rows per step (8 transposes, 1 evac, 8 matmuls)
            for r in range(rb0, rb0 + GB, 2):
                tp = psum.tile([128, 768], BF16, tag="ps")
                for j in range(4):
                    rr, half = r + j // 2, j % 2
                    c0 = 1 + 128 * half
                    o = 192 * j
                    nc.tensor.transpose(tp[:, o:o + 96],
                                        qv3[:, rr, c0:c0 + 128], ident_t[:])
                    nc.tensor.transpose(tp[:, o + 96:o + 192],
                                        kv3[:, rr, c0:c0 + 128], ident_t[:])
                ts = gsb.tile([128, 768], BF16, tag="ts")
                nc.vector.tensor_copy(ts[:], tp[:])
                for j in range(4):
                    o = 192 * j
                    st = (nstep == 0 and j == 0)
                    sp = (nstep == NSTEPS - 1 and j == 3)
                    nc.tensor.matmul(G12t_ps[:, 0:48], ts[:, o + 96:o + 144],
                                     ts[:, o + 48:o + 96], start=st, stop=sp,
                                     skip_group_check=True)
                    nc.tensor.matmul(G12t_ps[:, 48:96], ts[:, o + 144:o + 192],
                                     ts[:, o:o + 48], start=st, stop=sp,
                                     skip_group_check=True)
                nstep += 1
        nc.vector.tensor_copy(statspack[:, 0:96], G12t_ps[:])
        _gacc_cm.__exit__(None, None, None)
        _gsb_cm.__exit__(None, None, None)
        _junk_cm.__exit__(None, None, None)
        _warm_cm.__exit__(None, None, None)
        nc.vector.tensor_copy(statspack[:, 96:104], ssqk[0:48, :])
        nc.gpsimd.dma_start(statspack[:, 104:112], ssqk[48:96, :])
        _qk_cm.__exit__(None, None, None)

        # ---- stage E: single AllReduce over the 4 cores sharing a batch ----
        # stats [48, 112]: 0:48 gram1^T, 48:96 gram2^T, then 4-stripe partial
        # sumsq each for q1 (96:100), k1 (100:104), q2 (104:108), k2 (108:112)
        # AllGather + 3 local adds: ~2x cheaper than AllReduce on the wire
        nc.sync.dma_start(stats_in[:], statspack[:])
        nc.gpsimd.collective_compute(
            "AllGather", ALU.bypass,
            ins=[stats_in[:]], outs=[stats_ag[:]],
            replica_groups=[[0, 1, 2, 3], [4, 5, 6, 7]],
        )
        statsRg = small.tile([48, 4, 112], BF16, tag="statsRg")
        nc.sync.dma_start(statsRg[:], stats_ag[:].transpose([1, 0, 2]))
        statsR = small.tile([48, 112], F32, tag="statsR")
        sR01 = small.tile([48, 112], F32, tag="sR01")
        nc.vector.tensor_tensor(sR01[:], statsRg[:, 0], statsRg[:, 1], ALU.add)
        nc.vector.tensor_tensor(statsR[:], statsRg[:, 2], statsRg[:, 3], ALU.add)
        nc.vector.tensor_tensor(statsR[:], statsR[:], sR01[:], ALU.add)

        # residual window lands in the space qt/kt vacated
        _xb_cm = tc.tile_pool(name="pool_xb", bufs=1)
        xbp = _xb_cm.__enter__()
        xb = xbp.tile([96, NPIX_MID], BF16, tag="xb")
        xrf = xres[:].rearrange("c h w -> c (h w)")
        # chunked so the small stats DMA isn't stuck behind one 9us transfer
        XR = 11 * WP
        for c0 in range(0, NPIX_MID, XR):
            c1 = min(NPIX_MID, c0 + XR)
            nc.sync.dma_start(xb[:, c0:c1], xrf[:, c0:c1])
        nc.vector.tensor_copy(wub[:, 0:2], xb[0:2, 0:2])

        # ---- v conv emitted after the collective dispatch so PE overlaps it
        wv9 = w_qkvf_t[:, 2]
        for tl in _chunks(_ntiles(NPIX_MID), 2):
            pss = _conv9_chunk(nc, psum, wv9, xf8v, tl, 0, 96)
            for ps, (n0, nn) in zip(pss, tl):
                nc.vector.tensor_copy(vt[:, n0:n0 + nn], ps[:, :nn])
        # zero padded columns of v (q/k handled by AP exclusion)
        nc.vector.memset(vt[:].rearrange("c (h w) -> c h w", w=WP)[:, :, 0:1], 0.0)
        nc.vector.memset(vt[:].rearrange("c (h w) -> c h w", w=WP)[:, :, WP - 1:WP], 0.0)

        # ---- stage F: normalize, softmax, fused (mid @ A) ----
        # q/k carry the x64 fp8 weight scale; it cancels exactly in
        # gram * rsq_q * rsq_k.  v's scale is folded into w_mid (host /64).
        ssf = small.tile([48, 4], F32, tag="ssf")
        for i in range(4):
            nc.vector.reduce_sum(ssf[:, i:i + 1],
                                 statsR[:, 96 + 4 * i:100 + 4 * i],
                                 axis=mybir.AxisListType.X)
        rsq = small.tile([48, 4], F32, tag="rsq")
        nc.scalar.activation(rsq[:], ssf[:], AF.Sqrt)
        nc.vector.reciprocal(rsq[:], rsq[:])
        A_bd = small.tile([96, 96], BF16, tag="A_bd")
        nc.vector.memset(A_bd[:], 0.0)
        id48 = ident_t[0:48, 0:48]
        for gi in range(2):
            Bt = statsR[:, 0:48] if gi == 0 else statsR[:, 48:96]
            kcol, qcol, tcol = (1, 2, 0) if gi == 0 else (3, 0, 1)
            Bs = small.tile([48, 48], BF16, tag=f"Bs{gi}")
            nc.vector.tensor_scalar_mul(Bs[:], Bt, rsq[:, kcol:kcol + 1])
            ps_t = psum.tile([48, 48], BF16, tag="ps")
            nc.tensor.transpose(ps_t[:], Bs[:], id48)
            sc = small.tile([48, 1], F32, tag=f"sc{gi}")
            nc.vector.tensor_tensor(sc[:], rsq[:, qcol:qcol + 1],
                                    t12_t[:, tcol:tcol + 1], ALU.mult)
            Gw = small.tile([48, 48], F32, tag=f"Gw{gi}")
            nc.vector.scalar_tensor_tensor(Gw[:], ps_t[:], sc[:, 0:1],
                                           mneg_t[:], ALU.mult, ALU.add)
            # no max-subtraction: logits are cosine-sims * t (|t|~1), and the
            # -1e30 mask exponentiates to exactly 0 -- exp is overflow-safe
            E = small.tile([48, 48], F32, tag=f"E{gi}")
            nc.scalar.activation(E[:], Gw[:], AF.Exp)
            sm = small.tile([48, 1], F32, tag=f"sm{gi}")
            nc.vector.reduce_sum(sm[:], E[:], axis=mybir.AxisListType.X)
            nc.vector.reciprocal(sm[:], sm[:])
            A = small.tile([48, 48], BF16, tag=f"A{gi}")
            nc.vector.tensor_scalar_mul(A[:], E[:], sm[:, 0:1])
            if gi == 0:
                nc.vector.tensor_copy(A_bd[0:48, 0:48], A[:])
            else:
                nc.sync.dma_start(A_bd[48:96, 48:96], A[:])
        # F = A_bd^T @ w_mid  (so F^T = W_mid @ A: attention + mid in one mm)
        F_ps = psum.tile([96, 96], F32, tag="ps")
        nc.tensor.matmul(F_ps[:], A_bd[:], w_mid_t[:], start=True, stop=True)
        F_sb = small.tile([96, 96], BF16, tag="F_sb")
        nc.vector.tensor_copy(F_sb[:], F_ps[:])

        # ---- stage G: fused attn-apply + mid conv + residual -> x' ----
        # fp8 casts (ACT) are inlined per tile so stage H starts immediately;
        # edge-mask rows 0 / 65 gate only the first / last two casts.
        nc.vector.memset(x12f8[:, NPIX_MID:NPIX_MID + 2], 0.0)
        tiles = _ntiles(NPIX_MID)
        ncast = len(tiles)
        # residual folded into PSUM via identity matmul: the evacuation
        # becomes a pure copy (ACT) and the fp8 cast rides GPSIMD, so the
        # phase isn't paced by DVE psum-reads.  Chunked so F / identity
        # weights load once per chunk, not per tile.
        for tl in _chunks(list(enumerate(tiles)), 2):
            pss = [psum.tile([96, NT], F32, tag="ps", name=f"psc{j}")
                   for j in range(len(tl))]
            for ps, (i, (n0, nn)) in zip(pss, tl):
                nc.tensor.matmul(ps[:, :nn], F_sb[:], vt[:, n0:n0 + nn],
                                 start=True, stop=False,
                                 skip_group_check=True)
            for ps, (i, (n0, nn)) in zip(pss, tl):
                nc.tensor.matmul(ps[:, :nn], ident_t[:], xb[:, n0:n0 + nn],
                                 start=False, stop=True,
                                 skip_group_check=True)
            for ps, (i, (n0, nn)) in zip(pss, tl):
                nc.scalar.copy(x12p[:, n0:n0 + nn], ps[:, :nn])
                if i == 0:
                    nc.vector.tensor_scalar_mul(x12p[:, 0:WP], x12p[:, 0:WP],
                                                masks_t[:, 0:1])
                if i < ncast - 2:
                    nc.gpsimd.tensor_copy(x12f8[:, n0:n0 + nn],
                                          x12p[:, n0:n0 + nn])
        nc.vector.tensor_scalar_mul(x12p[:, (R_MID - 1) * WP:],
                                    x12p[:, (R_MID - 1) * WP:], masks_t[:, 1:2])
        for n0, nn in tiles[-2:]:
            nc.gpsimd.tensor_copy(x12f8[:, n0:n0 + nn], x12p[:, n0:n0 + nn])
        _xb_cm.__exit__(None, None, None)
        _v_cm.__exit__(None, None, None)
        _xf_cm.__exit__(None, None, None)

        # ---- stage H: folded stage-2 convs, gated products, output ----
        # po is emitted one block behind the convs so the PE never stalls on
        # the gg (DVE) product at block boundaries; the last blocks are
        # smaller to shorten the un-overlapped tail.
        BLOCKS = [(0, 16), (16, 16), (32, 16), (48, 8), (56, 8)]
        NBLK = len(BLOCKS)
        x12v = x12f8[:]
        with tc.tile_pool(name="blk", bufs=1) as bp, \
             tc.tile_pool(name="psum2", bufs=2, space="PSUM") as psum2:
            ggs = {}

            def convs_block(blk):
                r0, rows = BLOCKS[blk]
                ol = rows * WP
                in_off = r0 * WP
                gg = bp.tile([128, 3, 16 * WP], F8, tag=f"gg{blk % 2}")
                ggs[blk] = gg
                for m in range(3):
                    wE9 = w_pEf_t[:, m]
                    w129 = w_pE12f_t[:, m]
                    g_m = bp.tile([128, 16 * WP], BF16, tag=f"g{m}")
                    for tl in _chunks(_ntiles(ol), 3):
                        pss = _conv9_chunk(nc, psum, wE9, x12v, tl,
                                           in_off, 128)
                        for ps, (n0, nn) in zip(pss, tl):
                            nc.scalar.activation(g_m[:, n0:n0 + nn],
                                                 ps[:, :nn], AF.Gelu,
                                                 scale=1.0 / WS)
                    if blk < 3:
                        for tl in _chunks(_ntiles(ol), 2):
                            pss = _conv9_chunk(nc, psum2, w129, x12v, tl,
                                               in_off, 128)
                            # fused: gg = 8/WS * psum_e * gelu_g from PSUM
                            # (x8 so the fp8 gg feeds a DoubleRow po)
                            for ps, (n0, nn) in zip(pss, tl):
                                nc.vector.scalar_tensor_tensor(
                                    gg[:, m, n0:n0 + nn], ps[:, :nn],
                                    8.0 / WS, g_m[:, n0:n0 + nn],
                                    ALU.mult, ALU.mult)
                    else:
                        # tail blocks: keep DVE free for the outf adds --
                        # evacuate on ACT, multiply on GPSIMD (SBUF-only)
                        e_m = bp.tile([128, 8 * WP], BF16, tag="e_t")
                        for tl in _chunks(_ntiles(ol), 2):
                            pss = _conv9_chunk(nc, psum, w129, x12v, tl,
                                               in_off, 128)
                            for ps, (n0, nn) in zip(pss, tl):
                                nc.scalar.activation(e_m[:, n0:n0 + nn],
                                                     ps[:, :nn], AF.Copy,
                                                     scale=8.0 / WS)
                                nc.gpsimd.tensor_tensor(
                                    gg[:, m, n0:n0 + nn], e_m[:, n0:n0 + nn],
                                    g_m[:, n0:n0 + nn], ALU.mult)

            def po_block(blk):
                r0, rows = BLOCKS[blk]
                ol = rows * WP
                out_off = (r0 + 1) * WP
                gg = ggs.pop(blk)
                ggv = gg[:].rearrange("c m n -> c (m n)")
                GOL = 16 * WP
                outf = bp.tile([96, 16 * WP], BF16, tag=f"outf{blk % 2}")
                for tl in _chunks(_ntiles(ol), 2):
                    pss = [psum.tile([96, NT], F32, tag="ps", name=f"psc{j}")
                   for j in range(len(tl))]
                    for ps, (n0, nn) in zip(pss, tl):
                        rhs = _dr_rhs(ggv, n0, GOL, nn)
                        nc.tensor.matmul(ps[:, :nn], w_pof_t[:], rhs,
                                         start=True, stop=False, perf_mode=DR,
                                         skip_group_check=True)
                    for ps, (n0, nn) in zip(pss, tl):
                        nc.tensor.matmul(
                            ps[:, :nn], w_po2_t[:],
                            ggv[:, 2 * GOL + n0:2 * GOL + n0 + nn],
                            start=False, stop=True, skip_group_check=True)
                    # po weights carry xWS, gg carries x8 -> descale fused
                    for ps, (n0, nn) in zip(pss, tl):
                        nc.vector.scalar_tensor_tensor(
                            outf[:, n0:n0 + nn], ps[:, :nn], 1.0 / (8.0 * WS),
                            x12p[:, out_off + n0:out_off + n0 + nn],
                            ALU.mult, ALU.add)
                ov = outf[:].rearrange("c (h w) -> c h w", w=WP)
                nc.sync.dma_start(out_ext[:, r0:r0 + rows, :],
                                  ov[:, 0:rows, 1:1 + W])

            convs_block(0)
            for blk in range(1, NBLK):
                convs_block(blk)
                po_block(blk - 1)
            po_block(NBLK - 1)
        _x12_cm.__exit__(None, None, None)
    _legalize_waits(nc)
    return nc


_NC_CACHE = None


def _get_nc():
    global _NC_CACHE
    if _NC_CACHE is None:
        _NC_CACHE = build_graph()
    return _NC_CACHE


def _bf16(a):
    return np.ascontiguousarray(a.astype(ml_dtypes.bfloat16))


def _f8(a):
    return np.ascontiguousarray(
        np.clip(a, -240.0, 240.0).astype(ml_dtypes.float8_e4m3))


def _prep_weights(ins):
    dim = DIM
    out = {}
    # folded qkv: per group g and tap t, M_t = (diag(d_t) @ W)^T = W^T * d[cols]
    wq = np.zeros((3, 9, 96, 96), np.float32)
    for g in range(3):
        W1 = ins["qkv1_w"][g * dim:(g + 1) * dim, :, 0, 0]  # [48 out, 48 in]
        W2 = ins["qkv2_w"][g * dim:(g + 1) * dim, :, 0, 0]
        for pos, t in enumerate(TAP_ORDER):
            d1 = ins["qkv1_dw"][g * dim:(g + 1) * dim, 0, t // 3, t % 3]
            d2 = ins["qkv2_dw"][g * dim:(g + 1) * dim, 0, t // 3, t % 3]
            wq[g, pos, 0:48, 0:48] = W1.T * d1[None, :]
            wq[g, pos, 48:96, 48:96] = W2.T * d2[None, :]
    out["w_qkvf"] = _f8(np.transpose(wq, (2, 0, 1, 3)) * WS)

    Wp = ins["pE_w"][:, :, 0, 0]          # [384 out, 96 in]
    dpe = ins["pE_dw"][:, 0]              # [384, 3, 3]
    wpe = np.zeros((3, 9, 96, 128), np.float32)
    for pos, t in enumerate(TAP_ORDER):
        M = Wp.T * dpe[:, t // 3, t % 3][None, :]   # [96, 384]
        for m in range(3):
            wpe[m, pos] = M[:, 128 * m:128 * (m + 1)]
    out["w_pEf"] = _f8(np.transpose(wpe, (2, 0, 1, 3)) * WS)

    W12 = np.zeros((96, 384), np.float32)
    W12[0:48, 0:192] = ins["pE1_w"][:, :, 0, 0].T
    W12[48:96, 192:384] = ins["pE2_w"][:, :, 0, 0].T
    d12 = np.concatenate([ins["pE1_dw"][:, 0], ins["pE2_dw"][:, 0]], axis=0)
    w12 = np.zeros((3, 9, 96, 128), np.float32)
    for pos, t in enumerate(TAP_ORDER):
        M = W12 * d12[:, t // 3, t % 3][None, :]
        for m in range(3):
            w12[m, pos] = M[:, 128 * m:128 * (m + 1)]
    out["w_pE12f"] = _f8(np.transpose(w12, (2, 0, 1, 3)) * WS)

    wpo = np.concatenate([ins["po1_w"][:, :, 0, 0].T,
                          ins["po2_w"][:, :, 0, 0].T], axis=0)  # [384, 96]
    out["w_pof"] = _f8(np.stack([wpo[0:128], wpo[128:256]], axis=1) * WS)
    out["w_po2"] = _f8(wpo[256:384] * WS)

    wm = np.zeros((96, 96), np.float32)
    wm[0:48, 0:48] = ins["mid1_w"][:, :, 0, 0].T
    wm[48:96, 48:96] = ins["mid2_w"][:, :, 0, 0].T
    out["w_mid"] = _bf16(wm / WS)   # absorbs v's fp8 weight scale
    out["ident"] = _bf16(np.eye(96, dtype=np.float32))
    hb = np.repeat(np.arange(8), 6)
    out["mneg"] = np.where(hb[:, None] == hb[None, :], 0.0, -1e30).astype(np.float32)
    t12 = np.zeros((48, 2), np.float32)
    t12[:, 0] = np.repeat(ins["t1"][:, 0, 0], 6)
    t12[:, 1] = np.repeat(ins["t2"][:, 0, 0], 6)
    out["t12e"] = t12
    return out


def _shard_x(x1, x2, si):
    """x1/x2: [48, 256, 256] one batch -> (xres bf16 [96,66,258],
    xf8 fp8 [96, 68*258+2]) shards, zero padded."""
    shf = np.zeros((96, R_IN, WP), np.float32)
    r0 = 64 * si - 2
    lo, hi = max(0, r0), min(H, r0 + R_IN)
    shf[0:48, lo - r0:hi - r0, 1:1 + W] = x1[:, lo:hi, :]
    shf[48:96, lo - r0:hi - r0, 1:1 + W] = x2[:, lo:hi, :]
    xres = shf[:, 1:1 + R_MID].astype(ml_dtypes.bfloat16)
    xf8 = np.zeros((96, NPIX_IN + 2), np.float32)
    xf8[:, 0:NPIX_IN] = shf.reshape(96, NPIX_IN)
    return np.ascontiguousarray(xres), _f8(xf8)


LAST_EXEC_NS = None


def _kernel_device(**inputs):
    global LAST_EXEC_NS
    nc = _get_nc()
    wts = _prep_weights(inputs)
    x1 = np.asarray(inputs["x1"], np.float32)
    x2 = np.asarray(inputs["x2"], np.float32)
    in_maps = []
    for core in range(8):
        bi, si = core // 4, core % 4
        m = dict(wts)
        xres, xf8 = _shard_x(x1[bi], x2[bi], si)
        m["xres"] = xres
        m["xf8"] = xf8
        mk = np.ones((96, 2), np.float32)
        if si == 0:
            mk[:, 0] = 0.0
        if si == 3:
            mk[:, 1] = 0.0
        m["masks"] = mk
        in_maps.append(m)
    trace = bool(os.environ.get("KERNEL_TRACE"))
    res = run_bass_kernel_spmd(nc, in_maps, core_ids=list(range(8)),
                               trace=trace)
    if trace:
        LAST_EXEC_NS = res.exec_time_ns
    out = np.zeros((B, 2 * DIM, H, W), np.float32)
    for core in range(8):
        bi, si = core // 4, core % 4
        out[bi, :, 64 * si:64 * si + 64, :] = \
            np.asarray(res.results[core]["out"]).astype(np.float32)
    return out


if __name__ == "__main__":
    pass


# ---------- host fallback (pure numpy, exact) ----------
def _erf(x):
    # Abramowitz-Stegun 7.1.26, max abs err ~1.5e-7
    sgn = np.sign(x)
    ax = np.abs(x)
    t = 1.0 / (1.0 + 0.3275911 * ax)
    y = 1.0 - (((((1.061405429 * t - 1.453152027) * t) + 1.421413741) * t
                - 0.284496736) * t + 0.254829592) * t * np.exp(-ax * ax)
    return sgn * y


def _pw(x, w):
    return np.einsum("oc,bchw->bohw", w[:, :, 0, 0], x, optimize=True)


def _dw3(x, w):
    b, C, Hh, Ww = x.shape
    xp = np.zeros((b, C, Hh + 2, Ww + 2), x.dtype)
    xp[:, :, 1:-1, 1:-1] = x
    out = np.zeros_like(x)
    for di in range(3):
        for dj in range(3):
            out += w[None, :, 0, di, dj, None, None] * xp[:, :, di:di + Hh, dj:dj + Ww]
    return out


def _l2n(t):
    n = np.sqrt((t * t).sum(axis=-1, keepdims=True))
    return t / np.maximum(n, 1e-12)


def _gelu(x):
    return 0.5 * x * (1.0 + _erf(x / np.sqrt(2.0).astype(np.float32)))


def _kernel_host(x1, x2, t1, t2, qkv1_w, qkv1_dw, qkv2_w, qkv2_dw,
                 mid1_w, mid2_w, pE_w, pE_dw, pE1_w, pE1_dw,
                 pE2_w, pE2_dw, po1_w, po2_w):
    b, c, h, w = x1.shape
    heads = t1.shape[0]
    ch = c // heads

    def to_heads(t):
        return t.reshape(b, heads, ch, h * w)

    qkv1 = _dw3(_pw(x1, qkv1_w), qkv1_dw)
    q1, k1, v1 = np.split(qkv1, 3, axis=1)
    qkv2 = _dw3(_pw(x2, qkv2_w), qkv2_dw)
    q2, k2, v2 = np.split(qkv2, 3, axis=1)
    q1, k1, v1 = to_heads(q1), to_heads(k1), to_heads(v1)
    q2, k2, v2 = to_heads(q2), to_heads(k2), to_heads(v2)
    q1, k1 = _l2n(q1), _l2n(k1)
    q2, k2 = _l2n(q2), _l2n(k2)
    attn1 = np.einsum("bhcn,bhdn->bhcd", q2, k1, optimize=True) * t1[None]
    attn2 = np.einsum("bhcn,bhdn->bhcd", q1, k2, optimize=True) * t2[None]

    def sm(a):
        a = a - a.max(-1, keepdims=True)
        e = np.exp(a)
        return e / e.sum(-1, keepdims=True)

    out1 = np.einsum("bhcd,bhdn->bhcn", sm(attn1), v1, optimize=True).reshape(b, c, h, w)
    out2 = np.einsum("bhcd,bhdn->bhcn", sm(attn2), v2, optimize=True).reshape(b, c, h, w)
    x1 = x1 + _pw(out1, mid1_w)
    x2 = x2 + _pw(out2, mid2_w)
    out = np.concatenate([x1, x2], axis=1)
    g = _dw3(_pw(out, pE_w), pE_dw)
    g1, g2 = np.split(g, 2, axis=1)
    y1 = _gelu(g1) * _dw3(_pw(x1, pE1_w), pE1_dw)
    y2 = _gelu(g2) * _dw3(_pw(x2, pE2_w), pE2_dw)
    return (out + _pw(y1, po1_w) + _pw(y2, po2_w)).astype(np.float32)


def kernel(**inputs):
    inputs = {k: np.asarray(v, np.float32) for k, v in inputs.items()}
    if _HAVE_BASS and not os.environ.get("KERNEL_FORCE_HOST"):
        try:
            return _kernel_device(**inputs)
        except Exception as e:
            sys.stderr.write(f"[kernel] device path failed ({type(e).__name__}: {e}); "
                             "falling back to host compute\n")
    return _kernel_host(**inputs)
